# revision 10
# baseline (speedup 1.0000x reference)
"""BiQRNN (fo-pooling) Trainium2 kernel, v2 — all-bf16 dataflow.

Data-parallel over batch across 8 NeuronCores (2 batch rows per core).
Per direction: g = W @ x with bf16 weights/activations (fp32 PSUM accum,
T=1024 moving columns per matmul -> half the instruction count of fp32r
at the same 1 col/cycle PE rate), ACT tanh/sigmoid out of PSUM into bf16
gates, DVE tensor_tensor_scan (fp32 internal state) for
h_t = a_t*h_{t-1} + (1-a_t)*z_t chained across chunks, y = o*h on GpSimd,
Y stored bf16 and upcast on host. The backward direction runs the same
forward routine on a host-reversed copy of X.

Startup: the first chunk is 256 columns and its weight/X dependencies are
issued first, spread across five engine DMA queues, so the PE stream
starts ~6us in instead of waiting behind the bulk prefetch. The last
block tapers (1024/768/256) to shorten the post-matmul drain chain.
"""

import numpy as np
from ml_dtypes import bfloat16

import concourse.bacc as bacc
import concourse.mybir as mybir
import concourse.tile as tile
from concourse import bass_utils

SEQ, BATCH, D_IN, HID = 2048, 16, 512, 512
NCORES = 8
BPC = BATCH // NCORES  # batch rows per core

f32 = mybir.dt.float32
bf16 = mybir.dt.bfloat16
Alu = mybir.AluOpType
Act = mybir.ActivationFunctionType

KT = D_IN // 128   # contraction tiles
HT = HID // 128    # h tiles per gate
MT = 3 * HT        # m tiles (z, f, o)
T = 1024           # steady-state chunk (max bf16 moving operand)
T0 = 256           # taper chunk at stream head/tail


def build_nc():
    nc = bacc.Bacc("TRN2", target_bir_lowering=False, debug=False)
    XT = nc.dram_tensor("xt", [2, KT, 128, BPC * SEQ], bf16, kind="ExternalInput")
    WT = nc.dram_tensor("wt", [2, KT, 128, 3 * HID], bf16, kind="ExternalInput")
    BIAS = nc.dram_tensor("bias", [2, 128, MT], f32, kind="ExternalInput")
    Y = nc.dram_tensor("y", [2, HT, 128, BPC * SEQ], bf16, kind="ExternalOutput")

    with tile.TileContext(nc) as tc:
        with (
            tc.tile_pool(name="wpool", bufs=1) as wpool,
            tc.tile_pool(name="bpool", bufs=1) as bpool,
            tc.tile_pool(name="rhs_pool", bufs=2) as rhs_pool,
            tc.tile_pool(name="ps_pool", bufs=4, space="PSUM") as ps_pool,
            tc.tile_pool(name="gate_pool", bufs=12) as gate_pool,
            tc.tile_pool(name="h_pool", bufs=6) as h_pool,
            tc.tile_pool(name="y_pool", bufs=4) as y_pool,
        ):
            w_sb = [[None] * KT for _ in range(2)]
            b_sb = [None, None]

            def load_w(d, k, eng):
                w = wpool.tile([128, 3 * HID], bf16, name=f"w_{d}_{k}")
                eng.dma_start(w[:], WT.ap()[d, k])
                w_sb[d][k] = w

            def load_w_split3(d, k):
                # one weight tile fetched as 3 pieces on the 3 DMA-capable
                # queues so it lands no later than its single-piece peers
                w = wpool.tile([128, 3 * HID], bf16, name=f"w_{d}_{k}")
                q = 3 * HID // 3
                for p, eng in enumerate((nc.sync, nc.gpsimd, nc.scalar)):
                    eng.dma_start(
                        w[:, p * q : (p + 1) * q], WT.ap()[d, k, :, p * q : (p + 1) * q]
                    )
                w_sb[d][k] = w

            def load_bias(d, eng):
                bt = bpool.tile([128, MT], f32, name=f"b_{d}")
                eng.dma_start(bt[:], BIAS.ap()[d])
                b_sb[d] = bt

            def new_rhs():
                return rhs_pool.tile([128, KT, SEQ], bf16, name="rhs")

            def load_rhs(t, d, b, eng, k_lo=0, k_hi=KT, c0=0, c1=SEQ):
                for k in range(k_lo, k_hi):
                    eng.dma_start(
                        t[:, k, c0:c1], XT.ap()[d, k, :, b * SEQ + c0 : b * SEQ + c1]
                    )

            # --- startup: first-chunk deps first, spread across queues ---
            # first chunk needs W[d0, all k] + rhs(d0,b0)[:, :, :T0]; the
            # rhs tails and everything else follow behind.
            rhs0 = new_rhs()
            load_w(0, 0, nc.sync)
            load_w(0, 1, nc.gpsimd)
            load_w(0, 2, nc.scalar)
            load_w_split3(0, 3)
            load_bias(0, nc.scalar)
            load_rhs(rhs0, 0, 0, nc.sync, k_lo=0, k_hi=1, c1=T0)
            load_rhs(rhs0, 0, 0, nc.gpsimd, k_lo=1, k_hi=2, c1=T0)
            load_rhs(rhs0, 0, 0, nc.scalar, k_lo=2, k_hi=3, c1=T0)
            load_rhs(rhs0, 0, 0, nc.gpsimd, k_lo=3, k_hi=4, c1=T0)
            load_rhs(rhs0, 0, 0, nc.sync, k_lo=0, k_hi=1, c0=T0)
            load_rhs(rhs0, 0, 0, nc.gpsimd, k_lo=1, k_hi=2, c0=T0)
            load_rhs(rhs0, 0, 0, nc.scalar, k_lo=2, k_hi=3, c0=T0)
            load_rhs(rhs0, 0, 0, nc.gpsimd, k_lo=3, k_hi=4, c0=T0)

            rhs_next = [None]
            blocks = [(0, 0), (0, 1), (1, 0), (1, 1)]
            for bi, (d, b) in enumerate(blocks):
                if bi == 0:
                    rhs = rhs0
                    chunks = [T0, SEQ - T - T0, T]
                else:
                    rhs = rhs_next[0]
                if bi + 1 < len(blocks):
                    dn, bn = blocks[bi + 1]
                    rhs_next[0] = new_rhs()
                    load_rhs(rhs_next[0], dn, bn, nc.sync)
                if bi == len(blocks) - 1:
                    chunks = [T, SEQ - T - T0, T0]
                elif bi > 0:
                    chunks = [T, SEQ - T]

                hprev = None
                t0 = 0
                for ci, tl in enumerate(chunks):
                    if bi == 1:
                        # trickle the bw-direction constants in while the
                        # fw stream runs; startup traffic has drained by now
                        for k in range(ci * 2, min(ci * 2 + 2, KT)):
                            load_w(1, k, nc.gpsimd)
                        if ci == 0:
                            load_bias(1, nc.scalar)
                    hcur = [None] * HT
                    for hti in range(HT):
                        acts = []
                        for g in range(3):
                            m = g * HT + hti
                            ps = ps_pool.tile([128, T], f32, name="ps")
                            # ISA caps one matmul at 512 psum columns (one
                            # bank), so fill the 1024-wide tile in halves
                            for s0 in range(0, tl, 512):
                                sl = min(512, tl - s0)
                                for k in range(KT):
                                    nc.tensor.matmul(
                                        ps[:, s0 : s0 + sl],
                                        w_sb[d][k][:, m * 128 : (m + 1) * 128],
                                        rhs[:, k, t0 + s0 : t0 + s0 + sl],
                                        start=(k == 0),
                                        stop=(k == KT - 1),
                                    )
                            gt = gate_pool.tile(
                                [128, T], bf16, name=("zt", "at", "ot")[g]
                            )
                            nc.scalar.activation(
                                gt[:, :tl],
                                ps[:, :tl],
                                Act.Tanh if g == 0 else Act.Sigmoid,
                                bias=b_sb[d][:, m : m + 1],
                                scale=-1.0 if g == 1 else 1.0,
                            )
                            acts.append(gt)
                        zt, at, ot = acts
                        cp = gate_pool.tile([128, T], bf16, name="cp")
                        # cp = (a - 1) * z = -c. The DVE owns the
                        # latency-bound scans, so most cp's go to gpsimd —
                        # which lacks scalar_tensor_tensor, hence two plain
                        # tensor_tensors (a*z then -z) there.
                        if hti == 0:
                            nc.vector.scalar_tensor_tensor(
                                cp[:, :tl], at[:, :tl], 1.0, zt[:, :tl],
                                op0=Alu.subtract, op1=Alu.mult,
                            )
                        else:
                            tmp = gate_pool.tile([128, T], bf16, name="cptmp")
                            nc.gpsimd.tensor_tensor(
                                tmp[:, :tl], at[:, :tl], zt[:, :tl], op=Alu.mult
                            )
                            nc.gpsimd.tensor_tensor(
                                cp[:, :tl], tmp[:, :tl], zt[:, :tl], op=Alu.subtract
                            )
                        h = h_pool.tile([128, T], bf16, name="h")
                        init = 0.0 if ci == 0 else hprev[hti]
                        # h_t = a_t * h_{t-1} - cp_t
                        nc.vector.tensor_tensor_scan(
                            h[:, :tl], at[:, :tl], cp[:, :tl], init,
                            op0=Alu.mult, op1=Alu.subtract,
                        )
                        hcur[hti] = h[:, tl - 1 : tl]
                        yt = y_pool.tile([128, T], bf16, name="yt")
                        nc.vector.tensor_tensor(
                            yt[:, :tl], ot[:, :tl], h[:, :tl], op=Alu.mult
                        )
                        nc.sync.dma_start(
                            Y.ap()[d, hti, :, b * SEQ + t0 : b * SEQ + t0 + tl],
                            yt[:, :tl],
                        )
                    hprev = hcur
                    t0 += tl
    nc.compile()
    return nc


def prep_inputs(X, W_fw, b_fw, W_bw, b_bw):
    """Host-side shard/transpose/bf16-cast. Returns per-core in_maps."""
    WTa = np.empty((2, KT, 128, 3 * HID), bfloat16)
    BIAS = np.empty((2, 128, MT), np.float32)
    for d, (W, bvec) in enumerate(((W_fw, b_fw), (W_bw, b_bw))):
        WTa[d] = np.ascontiguousarray(W.T).reshape(KT, 128, 3 * HID).astype(bfloat16)
        bm = bvec.reshape(MT, 128).T.copy()  # [128, MT]
        bm[:, HT : 2 * HT] *= -1.0  # f-gate bias negated (a = sigmoid(-g - b))
        BIAS[d] = bm

    # one big [S,B,D] -> [D,B,S] transpose + bf16 cast, then per-core blocks
    XTa = (
        np.ascontiguousarray(np.transpose(X, (2, 1, 0)))
        .astype(bfloat16)
        .reshape(KT, 128, BATCH, SEQ)
    )
    in_maps = []
    for c in range(NCORES):
        xt = np.empty((2, KT, 128, BPC, SEQ), bfloat16)
        blk = XTa[:, :, c * BPC : (c + 1) * BPC, :]
        xt[0] = blk
        xt[1] = blk[..., ::-1]
        in_maps.append(
            {"xt": xt.reshape(2, KT, 128, BPC * SEQ), "wt": WTa, "bias": BIAS}
        )
    return in_maps


def assemble_output(results):
    """results: list of per-core {'y': [2, HT, 128, tok]} -> [SEQ, BATCH, 2*HID]."""
    out = np.empty((SEQ, BATCH, 2 * HID), np.float32)
    for c in range(NCORES):
        Yc = np.asarray(results[c]["y"]).astype(np.float32)
        for b in range(BPC):
            gb = c * BPC + b
            yf = Yc[0, :, :, b * SEQ : (b + 1) * SEQ].reshape(HID, SEQ)
            yb = Yc[1, :, :, b * SEQ : (b + 1) * SEQ].reshape(HID, SEQ)
            out[:, gb, :HID] = yf.T
            out[:, gb, HID:] = yb.T[::-1]
    return out


_NC_CACHE = {}


def _get_nc():
    if "nc" not in _NC_CACHE:
        _NC_CACHE["nc"] = build_nc()
    return _NC_CACHE["nc"]


def kernel(X, W_fw, b_fw, W_bw, b_bw, trace=False):
    X = np.asarray(X, np.float32)
    nc = _get_nc()
    in_maps = prep_inputs(
        X,
        np.asarray(W_fw, np.float32),
        np.asarray(b_fw, np.float32),
        np.asarray(W_bw, np.float32),
        np.asarray(b_bw, np.float32),
    )
    res = bass_utils.run_bass_kernel_spmd(
        nc, in_maps, core_ids=list(range(NCORES)), trace=trace
    )
    out = assemble_output(res.results)
    if trace:
        kernel.last_results = res
    return out
